# revision 1
# baseline (speedup 1.0000x reference)
"""Trainium2 Bass kernel for nn_ConceptFusionModule (8-core, 1 collective).

Math: softmax over a single key collapses the SDPA, so the module reduces to

    cw      = softmax(textN @ Wc.T, -1)           # (B*L, C)
    cr      = cw.T @ textN  (per batch)           # (B, C, D)
    v       = cr @ Wv.T                           # (B, C, D)
    fused   = blend@v + sig(g)*.3*var_c(v)        # (B, D)
    obd     = rmsnorm(fused)*nw @ Wo.T            # (B, D)
    out     = x + obd[:, None, :]                 # broadcast over N

Profiling showed the runtime's 8-core NEFF entry barrier spans a fixed
~46 us of core-launch skew, independent of kernel structure: anything a
core computes in its first ~50 us is hidden under the barrier, and every
collective op costs >=10 us of serial CC-stream time after it.  So this
version replicates the text chain (full cr from the full 8.4 MB text,
cheap, fully under the barrier) and keeps exactly ONE collective:

  - Wv: core k holds WvT[:, 256k:256k+256]  -> v/fused for its e-slice.
  - Wo: core k holds WoT[256k:256k+256, :]  -> partial z = (fused*nw)@WoT
    (z is linear in rmsnorm's input, so the global scale is applied after
    the reduce), plus partial sum(fused^2) as payload column 2048.
  - AllReduce (16.4 KB) of [z | ssq]; obd = z * rsqrt(ssq/D + eps) with
    the rsqrt folded into the PE row-broadcast selector.

Post-barrier critical path = AllReduce + sel-matmul broadcast + DVE adds
reading obd rows straight from PSUM + the 8.4 MB store stream.

Per-core HBM traffic: x 8.4 + out 8.4 + text 8.4 + Wv/8 2.1 + Wo/8 2.1
= ~29.5 MB, all loads overlapped with the entry barrier.
"""

import os

import numpy as np

import concourse.bacc as bacc
import concourse.bass as bass
import concourse.mybir as mybir
import concourse.tile as tile
from concourse import masks
from concourse.bass_utils import run_bass_kernel_spmd

F32 = mybir.dt.float32
F32R = mybir.dt.float32r

N_CORES = 8
B, N, L, D, C = 2, 4096, 256, 2048, 6
BL = B * L               # 512 text rows
LT = BL // 128           # 4 text l-tiles
ES = D // N_CORES        # 256-wide d/e slice per core
ROWS = B * N // N_CORES  # 1024 x rows per core (512 per batch)
HALF = ROWS // 2
KT = D // 128            # 16 contraction k-tiles
JW = ES // 128           # 2 k-tiles for the wot contraction
G = 2                    # 128-row tiles per x supertile
NST = ROWS // (128 * G)  # 4 supertiles
AX = mybir.AxisListType.X
AF = mybir.ActivationFunctionType
ADD = mybir.AluOpType.add
MUL = mybir.AluOpType.mult
RG = [list(range(N_CORES))]


def build_nc(is_surreal: bool) -> bacc.Bacc:
    nc = bacc.Bacc("TRN2", target_bir_lowering=False, debug=False,
                   num_devices=N_CORES)

    x_d = nc.dram_tensor("x_shard", [ROWS, D], F32, kind="ExternalInput")
    tN_d = nc.dram_tensor("tN", [BL, D], F32, kind="ExternalInput")
    tT_d = nc.dram_tensor("tT", [D, BL], F32, kind="ExternalInput")
    wct_d = nc.dram_tensor("WcT", [D, C], F32, kind="ExternalInput")
    wvt_d = nc.dram_tensor("wvt", [D, ES], F32, kind="ExternalInput")
    wot_d = nc.dram_tensor("wot", [ES, D], F32, kind="ExternalInput")
    bl_d = nc.dram_tensor("blend", [1, C], F32, kind="ExternalInput")
    sg_d = nc.dram_tensor("sg2", [2, 1], F32, kind="ExternalInput")
    nw_d = nc.dram_tensor("nw2", [2, ES], F32, kind="ExternalInput")
    out_d = nc.dram_tensor("out_shard", [ROWS, D], F32, kind="ExternalOutput")

    with tile.TileContext(nc) as tc:
        with (
            tc.tile_pool(name="pc", bufs=1) as pc,
            tc.tile_pool(name="pwrk", bufs=2) as pwrk,
            tc.tile_pool(name="dram", bufs=1, space="DRAM") as dram,
        ):
            # ---- persistent SBUF tiles ----
            tT_sb = pc.tile([128, KT, BL], F32R)
            tN_sb = pc.tile([128, LT, D], F32R)
            wct_sb = pc.tile([128, KT, C], F32R)
            wvt_sb = pc.tile([128, KT, ES], F32R)
            wot_sb = pc.tile([128, JW, D], F32R)
            bl_sb = pc.tile([1, C], F32)
            sg_sb = pc.tile([2, 1], F32)
            nw_sb = pc.tile([2, ES], F32)
            ident = pc.tile([128, 128], F32)
            eps_t = pc.tile([2, 1], F32)
            blendn = pc.tile([1, C], F32)
            blendn2 = pc.tile([1, 2 * C], F32)
            m12 = pc.tile([2 * C, 1], F32)
            m12c = pc.tile([2 * C, 1], F32)
            bd4 = pc.tile([2 * C, 4], F32)
            ones2 = pc.tile([2 * C, 2], F32)
            g3 = pc.tile([2, 1], F32)
            cwb4 = pc.tile([128, LT, 2 * C], F32R)
            crT_sb = pc.tile([128, KT, 2 * C], F32R)
            v_sb = pc.tile([2 * C, ES], F32)
            v2_sb = pc.tile([2 * C, ES], F32)
            fused = pc.tile([2, ES], F32)
            fy_sb = pc.tile([2, ES], F32)
            sqf = pc.tile([2, ES], F32)
            ssq = pc.tile([2, 1], F32)
            fyT = pc.tile([128, JW, 2], F32R)
            z_sb = pc.tile([2, D + 1], F32R)
            zr_sb = pc.tile([2, D + 1], F32R)
            ms = pc.tile([2, 1], F32)
            rs = pc.tile([2, 1], F32)
            sel0 = pc.tile([2, 128], F32)
            sel1 = pc.tile([2, 128], F32)
            sel0s = pc.tile([2, 128], F32R)
            sel1s = pc.tile([2, 128], F32R)
            bcz0 = pc.tile([128, D], F32)
            bcz1 = pc.tile([128, D], F32)

            # ---- internal DRAM (collective bounce buffers) ----
            dum_sb = pc.tile([1, 8], F32)
            dum_in = dram.tile([1, 8], F32)
            dum_out = dram.tile([N_CORES, 8], F32)
            ar2_in = dram.tile([2, D + 1], F32)
            ar2_out = dram.tile([2, D + 1], F32)

            # ---- warmup collective: pays the ncfw wake + first-op cost
            # inside the entry-barrier window so the real AllReduce hits
            # the warm CC stream ----
            nc.gpsimd.memset(dum_sb[:], 0.0)
            nc.gpsimd.dma_start(out=dum_in[:], in_=dum_sb[:])
            nc.gpsimd.collective_compute(
                "AllGather", mybir.AluOpType.bypass, replica_groups=RG,
                ins=[dum_in.opt()], outs=[dum_out.opt()])

            # ---- all loads on the sync HWDGE ring, critical-path first ----
            nc.sync.dma_start(out=tT_sb[:],
                              in_=tT_d.ap().rearrange("(j p) l -> p j l",
                                                      p=128).bitcast(F32R))
            nc.sync.dma_start(out=wct_sb[:],
                              in_=wct_d.ap().rearrange("(j p) c -> p j c",
                                                       p=128).bitcast(F32R))
            nc.sync.dma_start(out=tN_sb[:],
                              in_=tN_d.ap().rearrange("(g p) d -> p g d",
                                                      p=128).bitcast(F32R))
            nc.sync.dma_start(out=bl_sb[:], in_=bl_d.ap())
            nc.sync.dma_start(out=sg_sb[:], in_=sg_d.ap())
            nc.sync.dma_start(out=nw_sb[:], in_=nw_d.ap())
            nc.sync.dma_start(out=wvt_sb[:],
                              in_=wvt_d.ap().rearrange("(j p) e -> p j e",
                                                       p=128).bitcast(F32R))
            nc.sync.dma_start(out=wot_sb[:],
                              in_=wot_d.ap().rearrange("(j p) e -> p j e",
                                                       p=128).bitcast(F32R))
            px_cm = tc.tile_pool(name="px", bufs=1)
            px = px_cm.__enter__()
            xts = []
            for s in range(NST):
                xt = px.tile([128, G, D], F32, name=f"xst{s}")
                nc.sync.dma_start(
                    out=xt[:],
                    in_=x_d[128 * G * s:128 * G * (s + 1), :].rearrange(
                        "(g p) d -> p g d", p=128))
                xts.append(xt)

            # ---- constants ----
            masks.make_identity(nc, ident[:])
            nc.gpsimd.memset(eps_t[:], 1e-6)
            nc.gpsimd.memset(cwb4[:].bitcast(F32), 0.0)
            nc.gpsimd.memset(sel0[:], 0.0)
            nc.gpsimd.memset(sel0[0:1, :], 1.0)
            nc.vector.tensor_scalar(sel1[:], sel0[:], -1.0, 1.0,
                                    op0=MUL, op1=ADD)
            # m12 = [1]*C + [0]*C column; m12c its complement
            nc.gpsimd.memset(m12[:], 0.0)
            nc.gpsimd.memset(m12[0:C, 0:1], 1.0)
            nc.vector.tensor_scalar(m12c[:], m12[:], -1.0, 1.0,
                                    op0=MUL, op1=ADD)
            nc.vector.tensor_copy(ones2[:, 0:1], m12[:])
            nc.vector.tensor_copy(ones2[:, 1:2], m12c[:])
            nc.vector.tensor_copy(bd4[:, 2:3], m12[:])
            nc.vector.tensor_copy(bd4[:, 3:4], m12c[:])

            # ---- blend softmax + g3 + bd4 blend columns (tiny, early) ----
            nc.scalar.activation(blendn[:], bl_sb[:], AF.Exp)
            bsum = pwrk.tile([1, 1], F32)
            nc.vector.reduce_sum(bsum[:], blendn[:], axis=AX)
            brcp = pwrk.tile([1, 1], F32)
            nc.vector.reciprocal(brcp[:], bsum[:])
            nc.vector.tensor_scalar_mul(blendn[:], blendn[:], brcp[:])
            nc.vector.tensor_copy(blendn2[0:1, 0:C], blendn[:])
            nc.vector.tensor_copy(blendn2[0:1, C:2 * C], blendn[:])
            with tc.tile_pool(name="ps_bl", bufs=1, space="PSUM") as ps_bl:
                blt_ps = ps_bl.tile([2 * C, 1], F32)
                nc.tensor.transpose(blt_ps[:], blendn2[:], ident[0:1, 0:1])
                nc.vector.tensor_mul(bd4[:, 0:1], blt_ps[:], m12[:])
                nc.vector.tensor_mul(bd4[:, 1:2], blt_ps[:], m12c[:])
            if is_surreal:
                nc.scalar.activation(g3[:], sg_sb[:], AF.Sigmoid)
                nc.scalar.mul(g3[:], g3[:], 0.3 / (C - 1))

            # ---- logits per l-tile; softmax over C (logits are O(1)-scale
            # with the 0.02 weight init, so exp() is safe without the max
            # subtraction) -> block-diagonal cluster weights cwb4 ----
            with tc.tile_pool(name="ps_lg", bufs=1, space="PSUM") as ps_lg:
                lg_ps = [ps_lg.tile([128, C], F32, name=f"lg{lt}",
                                    tag=f"lg{lt}") for lt in range(LT)]
                for lt in range(LT):
                    for j in range(KT):
                        nc.tensor.matmul(
                            lg_ps[lt][:],
                            tT_sb[:, j, 128 * lt:128 * (lt + 1)],
                            wct_sb[:, j, :],
                            start=(j == 0), stop=(j == KT - 1))
                for lt in range(LT):
                    e_sb = pwrk.tile([128, C], F32, name=f"e{lt}", tag="e")
                    nc.scalar.activation(e_sb[:], lg_ps[lt][:], AF.Exp)
                    ssum = pwrk.tile([128, 1], F32, name=f"ss{lt}", tag="ss")
                    nc.vector.reduce_sum(ssum[:], e_sb[:], axis=AX)
                    srcp = pwrk.tile([128, 1], F32, name=f"sr{lt}", tag="sr")
                    nc.vector.reciprocal(srcp[:], ssum[:])
                    off = 0 if lt < LT // 2 else C
                    nc.vector.tensor_scalar_mul(cwb4[:, lt, off:off + C],
                                                e_sb[:], srcp[:])

            # ---- full crT[d, b*c] = textN.T @ cwb4 (contraction over l,
            # j-sequential accumulation chains) ----
            with tc.tile_pool(name="ps_cp", bufs=1, space="PSUM") as ps_cp:
                crT_ps = ps_cp.tile([128, KT, 2 * C], F32)
                for j in range(KT):
                    for lt in range(LT):
                        nc.tensor.matmul(
                            crT_ps[:, j, :],
                            tN_sb[:, lt, 128 * j:128 * (j + 1)],
                            cwb4[:, lt, :],
                            start=(lt == 0), stop=(lt == LT - 1))
                nc.vector.tensor_copy(crT_sb[:], crT_ps[:])

            # ---- v[b*c, e-slice] = crT.T @ WvT cols ----
            with tc.tile_pool(name="ps_v", bufs=1, space="PSUM") as ps_v:
                v_ps = ps_v.tile([2 * C, ES], F32)
                for j in range(KT):
                    nc.tensor.matmul(v_ps[:], crT_sb[:, j, :], wvt_sb[:, j, :],
                                     start=(j == 0), stop=(j == KT - 1))
                nc.vector.tensor_copy(v_sb[:], v_ps[:])
                if is_surreal:
                    nc.vector.tensor_mul(v2_sb[:], v_sb[:], v_ps[:])

            # ---- fused[b, e-slice] = blend@v + g3*(s2 - s1^2/C) ----
            with tc.tile_pool(name="ps_d", bufs=1, space="PSUM") as ps_d:
                fl_ps = ps_d.tile([2, ES], F32)
                nc.tensor.matmul(fl_ps[:], bd4[:, 0:2], v_sb[:],
                                 start=True, stop=True)
                if is_surreal:
                    s1_ps = ps_d.tile([2, ES], F32)
                    nc.tensor.matmul(s1_ps[:], bd4[:, 2:4], v_sb[:],
                                     start=True, stop=True)
                    s2_ps = ps_d.tile([2, ES], F32)
                    nc.tensor.matmul(s2_ps[:], ones2[:], v2_sb[:],
                                     start=True, stop=True)
                    t1 = pwrk.tile([2, ES], F32)
                    nc.scalar.activation(t1[:], s1_ps[:], AF.Square)
                    t2 = pwrk.tile([2, ES], F32)
                    nc.vector.scalar_tensor_tensor(
                        t2[:], t1[:], -1.0 / C, s2_ps[:], op0=MUL, op1=ADD)
                    nc.vector.scalar_tensor_tensor(
                        fused[:], t2[:], g3[0:2, 0:1], fl_ps[:],
                        op0=MUL, op1=ADD)
                else:
                    nc.vector.tensor_copy(fused[:], fl_ps[:])

            # ---- fy = fused * nw; partial ssq -> payload column D ----
            nc.vector.tensor_mul(fy_sb[:], fused[:], nw_sb[:])
            nc.vector.tensor_mul(sqf[:], fused[:], fused[:])
            nc.vector.reduce_sum(ssq[:], sqf[:], axis=AX)
            nc.vector.tensor_copy(z_sb[:, D:D + 1], ssq[:])

            # ---- fyT[d-tile, b] via PE transpose ----
            with tc.tile_pool(name="ps_tr", bufs=2, space="PSUM") as ps_tr:
                for j in range(JW):
                    tp = ps_tr.tile([128, 2], F32, name=f"tp{j}", tag="tp")
                    nc.tensor.transpose(tp[:],
                                        fy_sb[:, 128 * j:128 * (j + 1)],
                                        ident[0:2, 0:2])
                    nc.vector.tensor_copy(fyT[:, j, :], tp[:])

            # ---- partial z[b, e'] = fyT.T @ WoT rows; the one AllReduce ----
            with tc.tile_pool(name="ps_z", bufs=1, space="PSUM") as ps_z:
                zps = [ps_z.tile([2, 512], F32, name=f"zp{ch}", tag=f"zp{ch}")
                       for ch in range(D // 512)]
                for j in range(JW):
                    for ch in range(D // 512):
                        nc.tensor.matmul(zps[ch][:],
                                         fyT[:, j, :],
                                         wot_sb[:, j, 512 * ch:512 * (ch + 1)],
                                         start=(j == 0), stop=(j == JW - 1))
                for ch in range(D // 512):
                    nc.vector.tensor_copy(z_sb[:, 512 * ch:512 * (ch + 1)],
                                          zps[ch][:])
            nc.scalar.dma_start(out=ar2_in[:], in_=z_sb[:].bitcast(F32))
            nc.gpsimd.collective_compute(
                "AllReduce", ADD, replica_groups=RG,
                ins=[ar2_in.opt()], outs=[ar2_out.opt()])
            nc.scalar.dma_start(out=zr_sb[:], in_=ar2_out[:].bitcast(F32R))

            # ---- rs = rsqrt(ssq/D + eps), folded into the sel rows ----
            nc.scalar.activation(ms[:], zr_sb[:, D:D + 1].bitcast(F32),
                                 AF.Sqrt, bias=eps_t[:], scale=1.0 / D)
            nc.vector.reciprocal(rs[:], ms[:])
            nc.vector.tensor_scalar_mul(sel0s[:], sel0[:], rs[:])
            nc.vector.tensor_scalar_mul(sel1s[:], sel1[:], rs[:])

            # ---- broadcast obd rows to 128 partitions (PSUM-resident) and
            # do the only O(N) work: out = x + obd[b] ----
            with tc.tile_pool(name="ps_bc", bufs=1, space="PSUM") as ps_bc:
                bc_ps = [ps_bc.tile([128, D], F32, name=f"bc{hb}",
                                    tag=f"bc{hb}") for hb in range(2)]
                for hb, sel in ((0, sel0s), (1, sel1s)):
                    for ch in range(D // 512):
                        nc.tensor.matmul(bc_ps[hb][:, 512 * ch:512 * (ch + 1)],
                                         sel[:],
                                         zr_sb[:, 512 * ch:512 * (ch + 1)],
                                         start=True, stop=True)
                # SBUF copies of the broadcast rows for the gpsimd adds
                # (gpsimd has no PSUM access); scalar engine does them.
                nc.scalar.activation(bcz0[:], bc_ps[0][:], AF.Copy)
                nc.scalar.activation(bcz1[:], bc_ps[1][:], AF.Copy)
                rings = [nc.scalar, nc.sync, nc.gpsimd, nc.scalar]
                for s in range(NST):
                    hb = 0 if s < NST // 2 else 1
                    for g in range(G):
                        t_idx = s * G + g
                        if t_idx % 4 == 3:
                            bcz = bcz0 if hb == 0 else bcz1
                            nc.gpsimd.tensor_add(xts[s][:, g, :],
                                                 xts[s][:, g, :], bcz[:])
                        else:
                            nc.vector.tensor_add(xts[s][:, g, :],
                                                 xts[s][:, g, :],
                                                 bc_ps[hb][:])
                    rings[s].dma_start(
                        out=out_d[128 * G * s:128 * G * (s + 1), :].rearrange(
                            "(g p) d -> p g d", p=128),
                        in_=xts[s][:])
            px_cm.__exit__(None, None, None)

    nc.compile()
    return nc


def prep_inputs(x, text_emb, Wc, Wv, Wo, blend_weights, surreal_gate,
                norm_weight):
    """Host-side layout prep (slice/transpose/replicate only)."""
    f = np.float32
    textN = np.ascontiguousarray(np.asarray(text_emb, f).reshape(BL, D))
    textT = np.ascontiguousarray(textN.T)
    WcT = np.ascontiguousarray(np.asarray(Wc, f).T)
    WvT = np.ascontiguousarray(np.asarray(Wv, f).T)
    WoT = np.ascontiguousarray(np.asarray(Wo, f).T)
    nw = np.asarray(norm_weight, f)
    blend = np.ascontiguousarray(np.asarray(blend_weights, f).reshape(1, C))
    sg2 = np.broadcast_to(np.asarray(surreal_gate, f).reshape(1, 1),
                          (2, 1)).copy()
    in_maps = []
    for k in range(N_CORES):
        es = slice(ES * k, ES * (k + 1))
        xs = np.concatenate(
            [x[0, HALF * k:HALF * (k + 1), :], x[1, HALF * k:HALF * (k + 1), :]],
            axis=0).astype(f)
        in_maps.append({
            "x_shard": np.ascontiguousarray(xs),
            "tN": textN,
            "tT": textT,
            "WcT": WcT,
            "wvt": np.ascontiguousarray(WvT[:, es]),
            "wot": np.ascontiguousarray(WoT[es, :]),
            "blend": blend,
            "sg2": sg2,
            "nw2": np.broadcast_to(nw[es][None, :], (2, ES)).copy(),
        })
    return in_maps


_CACHE = {}


def kernel(x, text_emb, Wc, Wq, Wk, Wv, Wo, blend_weights, surreal_gate,
           norm_weight, is_surreal, _collect=None):
    surreal = bool(int(np.asarray(is_surreal)))
    key = ("nc", surreal)
    if key not in _CACHE:
        _CACHE[key] = build_nc(surreal)
    nc = _CACHE[key]

    in_maps = prep_inputs(x, text_emb, Wc, Wv, Wo, blend_weights,
                          surreal_gate, norm_weight)
    res = run_bass_kernel_spmd(
        nc, in_maps, core_ids=list(range(N_CORES)),
        trace=os.environ.get("KERNEL_TRACE", "0") == "1",
    )
    if _collect is not None:
        _collect.append(res)

    out = np.empty((B, N, D), np.float32)
    for k in range(N_CORES):
        shard = res.results[k]["out_shard"]
        out[0, HALF * k:HALF * (k + 1), :] = shard[:HALF]
        out[1, HALF * k:HALF * (k + 1), :] = shard[HALF:]
    return out

